# revision 7
# baseline (speedup 1.0000x reference)
"""Block-sparse 3-layer MLP on 8 Trainium2 NeuronCores.

Reference computation (fp32):
    h1 = relu(x @ (W1*expand(mask1)).T + b1)       x:[B,2048] W1:[4096,2048]
    h2 = relu(h1 @ (W2*expand(mask2)).T + b2)      W2:[4096,4096]
    out = h2 @ Wo.T + bo                           Wo:[1024,4096] -> [B,1024]

Strategy: data-parallel over the batch (B=8192 -> 1024 rows/core), no
collectives. Masks are applied to the weights on the host (free) and all
matmuls run dense on the PE array in bf16 (1 cycle/row, same as f32r, but
half the DMA traffic and FWL-eligible 128-col weight loads).

Per core, three sequential phases (activations stay feature-major
[features, batch] in SBUF; biases are per-partition):
  L1: x (bf16, resident) @ W1 panels -> h1 (32 tiles [128,1024] bf16).
  L2: h1 @ W2 panels -> h2 (32 tiles, resident).
  L3: h2 @ Wo panels, accumulated over all 32 k-tiles in PSUM (one
      [128,1024] psum tile per output tile), bias via ACT -> DMA out.
No DVE work on the critical path; ACT only evicts PSUM once per tile.

Weight panels stream in [128,512] pieces round-robined across the SP and
gpsimd DMA queues; pool depths give two-m-tile lookahead. The first x
tiles are split into small chunks so the PE's first matmuls start early.
"""

import sys

sys.path.insert(0, "/opt/trn_rl_repo")

import numpy as np

from concourse import bacc, mybir, tile
from concourse.bass_utils import run_bass_kernel_spmd

F32 = mybir.dt.float32
BF16 = mybir.dt.bfloat16
RELU = mybir.ActivationFunctionType.Relu
IDENT = mybir.ActivationFunctionType.Identity

N_CORES = 8
TILE = 32  # block-sparse tile size of the masks
P = 128  # partitions
SW = 512  # psum strip width (max moving free dim)


def _build(nc, d_in, d_h, d_out, bc):
    kt1 = d_in // P  # 16 k-tiles in layer 1
    mt1 = d_h // P  # 32 m-tiles of h1
    mt2 = d_h // P  # 32 m-tiles of h2 (== k-tiles of layer 3)
    mot = d_out // P  # 8 m-tiles of out
    ns = bc // SW  # strips per row of tiles

    xt_d = nc.dram_tensor("xt", [kt1, P, bc], BF16, kind="ExternalInput")
    w1_d = nc.dram_tensor("w1", [mt1, P, d_in], BF16, kind="ExternalInput")
    b1_d = nc.dram_tensor("b1", [P, mt1], F32, kind="ExternalInput")
    w2_d = nc.dram_tensor("w2", [mt2, P, d_h], BF16, kind="ExternalInput")
    b2_d = nc.dram_tensor("b2", [P, mt2], F32, kind="ExternalInput")
    wo_d = nc.dram_tensor("wo", [mot, P, d_h], BF16, kind="ExternalInput")
    bo_d = nc.dram_tensor("bo", [P, mot], F32, kind="ExternalInput")
    out_d = nc.dram_tensor("out", [mot, P, bc], F32, kind="ExternalOutput")

    engines = [nc.sync, nc.gpsimd]
    ecnt = [0]

    def eng():
        e = engines[ecnt[0] % len(engines)]
        ecnt[0] += 1
        return e

    with tile.TileContext(nc) as tc:
        with (
            tc.tile_pool(name="bias", bufs=1) as bias_pool,
            tc.tile_pool(name="h1", bufs=1) as h1_pool,
            tc.tile_pool(name="h2", bufs=1) as h2_pool,
            tc.tile_pool(name="ps", bufs=3, space="PSUM") as ps_pool,
            tc.tile_pool(name="w2p", bufs=16) as w2_pool,
            tc.tile_pool(name="op", bufs=2) as o_pool,
        ):
            b1_sb = bias_pool.tile([P, mt1], F32, tag="b1")
            b2_sb = bias_pool.tile([P, mt2], F32, tag="b2")
            bo_sb = bias_pool.tile([P, mot], F32, tag="bo")
            nc.sync.dma_start(out=b1_sb[:], in_=b1_d[:])
            nc.sync.dma_start(out=b2_sb[:], in_=b2_d[:])
            nc.sync.dma_start(out=bo_sb[:], in_=bo_d[:])

            h1 = [
                h1_pool.tile([P, bc], BF16, name=f"h1_{i}", tag=f"h1_{i}")
                for i in range(mt1)
            ]
            h2 = [
                h2_pool.tile([P, bc], BF16, name=f"h2_{i}", tag=f"h2_{i}")
                for i in range(mt2)
            ]

            def load_panel(pool, dram, idx, cols, tag):
                """Stream dram[idx] ([P, cols]) in SW-wide chunks."""
                tiles = []
                for h in range(cols // SW):
                    t = pool.tile([P, SW], BF16, tag=tag)
                    eng().dma_start(
                        out=t[:], in_=dram[idx][:, h * SW : (h + 1) * SW]
                    )
                    tiles.append(t)
                return tiles

            # ---------------- Layer 1 ----------------
            with (
                tc.tile_pool(name="xtp", bufs=1) as xt_pool,
                tc.tile_pool(name="w1p", bufs=8) as w1_pool,
            ):
                # x: first tiles in fine chunks so the first matmuls are
                # gated on as little DMA as possible; spread across queues
                xt = [
                    xt_pool.tile([P, bc], BF16, name=f"xt_{k}", tag=f"xt_{k}")
                    for k in range(kt1)
                ]
                for k in range(kt1):
                    nchunk = 4 if k < 2 else 2
                    cw = bc // nchunk
                    for h in range(nchunk):
                        eng().dma_start(
                            out=xt[k][:, h * cw : (h + 1) * cw],
                            in_=xt_d[k][:, h * cw : (h + 1) * cw],
                        )

                w1h0 = load_panel(w1_pool, w1_d, 0, d_in, "w1t")

                w2pre = None
                for mt in range(mt1):
                    w1h = w1h0 if mt == 0 else load_panel(
                        w1_pool, w1_d, mt, d_in, "w1t"
                    )
                    ps = ps_pool.tile([P, bc], F32, name="ps1", tag="ps")
                    for kt in range(kt1):
                        h, r = divmod(kt, SW // P)
                        for n in range(ns):
                            nc.tensor.matmul(
                                ps[:, n * SW : (n + 1) * SW],
                                w1h[h][:, r * P : (r + 1) * P],
                                xt[kt][:, n * SW : (n + 1) * SW],
                                start=(kt == 0),
                                stop=(kt == kt1 - 1),
                            )
                    nc.scalar.activation(
                        h1[mt][:], ps[:], RELU, bias=b1_sb[:, mt : mt + 1]
                    )
                    if mt == mt1 - 2:
                        # prefetch W2's first panel while L1 finishes
                        w2pre = load_panel(w2_pool, w2_d, 0, d_h, "w2t")

            # ---------------- Layers 2 + 3 ----------------
            with tc.tile_pool(name="wop", bufs=16) as wo_pool:
                wopre = None
                for mt in range(mt2):
                    w2h = w2pre if mt == 0 else load_panel(
                        w2_pool, w2_d, mt, d_h, "w2t"
                    )
                    ps = ps_pool.tile([P, bc], F32, name="ps2", tag="ps")
                    for kt in range(mt1):
                        h, r = divmod(kt, SW // P)
                        for n in range(ns):
                            nc.tensor.matmul(
                                ps[:, n * SW : (n + 1) * SW],
                                w2h[h][:, r * P : (r + 1) * P],
                                h1[kt][:, n * SW : (n + 1) * SW],
                                start=(kt == 0),
                                stop=(kt == mt1 - 1),
                            )
                    nc.scalar.activation(
                        h2[mt][:], ps[:], RELU, bias=b2_sb[:, mt : mt + 1]
                    )
                    if mt == mt2 - 2:
                        # prefetch Wo's first panel while L2 finishes
                        wopre = load_panel(wo_pool, wo_d, 0, d_h, "wot")

                for mo in range(mot):
                    woh = wopre if mo == 0 else load_panel(
                        wo_pool, wo_d, mo, d_h, "wot"
                    )
                    ps = ps_pool.tile([P, bc], F32, name="ps3", tag="ps")
                    for kt in range(mt2):
                        h, r = divmod(kt, SW // P)
                        for n in range(ns):
                            nc.tensor.matmul(
                                ps[:, n * SW : (n + 1) * SW],
                                woh[h][:, r * P : (r + 1) * P],
                                h2[kt][:, n * SW : (n + 1) * SW],
                                start=(kt == 0),
                                stop=(kt == mt2 - 1),
                            )
                    ot = o_pool.tile([P, bc], F32, tag="ot")
                    nc.scalar.activation(
                        ot[:], ps[:], IDENT, bias=bo_sb[:, mo : mo + 1]
                    )
                    nchunk = 8
                    cw = bc // nchunk
                    for h in range(nchunk):
                        eng().dma_start(
                            out=out_d[mo][:, h * cw : (h + 1) * cw],
                            in_=ot[:, h * cw : (h + 1) * cw],
                        )

    nc.compile()
    return nc


def _expand_mask(mask, t=TILE):
    return np.repeat(np.repeat(np.asarray(mask, dtype=bool), t, axis=0), t, axis=1)


def _pack_lhsT(w, d_m, d_k):
    """[d_m, d_k] weights -> [d_m/P, P, d_k] panels.

    panel[mt, i, kt*P + j] = w[mt*P + j, kt*P + i], so each [P, P] slice of a
    panel is a ready-to-use lhsT block (partition dim = contraction dim).
    """
    mt, kt = d_m // P, d_k // P
    return np.ascontiguousarray(
        w.reshape(mt, P, kt, P).transpose(0, 3, 2, 1).reshape(mt, P, d_k)
    )


def _pack_bias(b):
    n = b.shape[0] // P
    return np.ascontiguousarray(b.reshape(n, P).T)


def _run(x, w1e, b1, w2e, b2, wo, bo, d_in, d_h, d_out, n_cores=N_CORES, trace=False):
    b = x.shape[0]
    bc = b // n_cores

    nc = bacc.Bacc(
        "TRN2", target_bir_lowering=False, debug=False, num_devices=n_cores
    )
    _build(nc, d_in, d_h, d_out, bc)

    bf16 = mybir.dt.np(BF16)

    def cvt(a):
        return np.ascontiguousarray(a.astype(bf16))

    shared = {
        "w1": cvt(_pack_lhsT(w1e, d_h, d_in)),
        "b1": _pack_bias(b1),
        "w2": cvt(_pack_lhsT(w2e, d_h, d_h)),
        "b2": _pack_bias(b2),
        "wo": cvt(_pack_lhsT(wo, d_out, d_h)),
        "bo": _pack_bias(bo),
    }
    in_maps = []
    for c in range(n_cores):
        xc = np.ascontiguousarray(x[c * bc : (c + 1) * bc].T).reshape(
            d_in // P, P, bc
        )
        in_maps.append({"xt": cvt(xc), **shared})

    res = run_bass_kernel_spmd(
        nc, in_maps, core_ids=list(range(n_cores)), trace=trace
    )
    outs = []
    for c in range(n_cores):
        outs.append(res.results[c]["out"].reshape(d_out, bc))
    full = np.concatenate(outs, axis=1)  # [d_out, B]
    return np.ascontiguousarray(full.T), res


def kernel(x, W1, b1, W2, b2, Wo, bo, mask1, mask2):
    x = np.asarray(x, dtype=np.float32)
    w1e = np.asarray(W1, dtype=np.float32) * _expand_mask(mask1)
    w2e = np.asarray(W2, dtype=np.float32) * _expand_mask(mask2)
    out, _ = _run(
        x,
        w1e,
        np.asarray(b1, np.float32),
        w2e,
        np.asarray(b2, np.float32),
        np.asarray(Wo, np.float32),
        np.asarray(bo, np.float32),
        d_in=2048,
        d_h=4096,
        d_out=1024,
    )
    return out


# revision 9
# speedup vs baseline: 1.1936x; 1.1936x over previous
"""Block-sparse 3-layer MLP on 8 Trainium2 NeuronCores.

Reference computation (fp32):
    h1 = relu(x @ (W1*expand(mask1)).T + b1)       x:[B,2048] W1:[4096,2048]
    h2 = relu(h1 @ (W2*expand(mask2)).T + b2)      W2:[4096,4096]
    out = h2 @ Wo.T + bo                           Wo:[1024,4096] -> [B,1024]

Strategy: data-parallel over the batch (B=8192 -> 1024 rows/core), no
collectives. Masks are applied to the weights on the host (free) and all
matmuls run dense on the PE array in bf16 (1 cycle/row, same as f32r, but
half the DMA traffic and FWL-eligible 128-col weight loads).

Per core, three sequential phases (activations stay feature-major
[features, batch] in SBUF; biases are per-partition):
  L1: x (bf16, resident) @ W1 panels -> h1 (32 tiles [128,1024] bf16).
  L2: h1 @ W2 panels -> h2 (32 tiles, resident).
  L3: h2 @ Wo panels, accumulated over all 32 k-tiles in PSUM (one
      [128,1024] psum tile per output tile), bias via ACT -> DMA out.
No DVE work on the critical path; ACT only evicts PSUM once per tile.

Weight panels stream in [128,512] pieces round-robined across the SP and
gpsimd DMA queues; pool depths give two-m-tile lookahead. The first x
tiles are split into small chunks so the PE's first matmuls start early.
"""

import sys

sys.path.insert(0, "/opt/trn_rl_repo")

import numpy as np

from concourse import bacc, mybir, tile
from concourse.bass_utils import run_bass_kernel_spmd

F32 = mybir.dt.float32
BF16 = mybir.dt.bfloat16
RELU = mybir.ActivationFunctionType.Relu
IDENT = mybir.ActivationFunctionType.Identity

N_CORES = 8
TILE = 32  # block-sparse tile size of the masks
P = 128  # partitions
SW = 512  # psum strip width (max moving free dim)


def _build(nc, d_in, d_h, d_out, bc):
    kt1 = d_in // P  # 16 k-tiles in layer 1
    mt1 = d_h // P  # 32 m-tiles of h1
    mt2 = d_h // P  # 32 m-tiles of h2 (== k-tiles of layer 3)
    mot = d_out // P  # 8 m-tiles of out
    ns = bc // SW  # strips per row of tiles

    xt_d = nc.dram_tensor("xt", [kt1, P, bc], BF16, kind="ExternalInput")
    w1_d = nc.dram_tensor("w1", [mt1, P, d_in], BF16, kind="ExternalInput")
    b1_d = nc.dram_tensor("b1", [P, mt1], F32, kind="ExternalInput")
    w2_d = nc.dram_tensor("w2", [mt2, P, d_h], BF16, kind="ExternalInput")
    b2_d = nc.dram_tensor("b2", [P, mt2], F32, kind="ExternalInput")
    wo_d = nc.dram_tensor("wo", [mot, P, d_h], BF16, kind="ExternalInput")
    bo_d = nc.dram_tensor("bo", [P, mot], F32, kind="ExternalInput")
    out_d = nc.dram_tensor("out", [mot, P, bc], F32, kind="ExternalOutput")

    engines = [nc.sync, nc.gpsimd]
    ecnt = [0]

    def eng():
        e = engines[ecnt[0] % len(engines)]
        ecnt[0] += 1
        return e

    with tile.TileContext(nc) as tc:
        with (
            tc.tile_pool(name="bias", bufs=1) as bias_pool,
            tc.tile_pool(name="h1", bufs=1) as h1_pool,
            tc.tile_pool(name="h2", bufs=1) as h2_pool,
            tc.tile_pool(name="ps", bufs=3, space="PSUM") as ps_pool,
            tc.tile_pool(name="w2p", bufs=16) as w2_pool,
            tc.tile_pool(name="op", bufs=2) as o_pool,
        ):
            b1_sb = bias_pool.tile([P, mt1], F32, tag="b1")
            b2_sb = bias_pool.tile([P, mt2], F32, tag="b2")
            bo_sb = bias_pool.tile([P, mot], F32, tag="bo")

            h1 = [
                h1_pool.tile([P, bc], BF16, name=f"h1_{i}", tag=f"h1_{i}")
                for i in range(mt1)
            ]
            h2 = [
                h2_pool.tile([P, bc], BF16, name=f"h2_{i}", tag=f"h2_{i}")
                for i in range(mt2)
            ]

            def load_panel(pool, dram, idx, cols, tag):
                """Stream dram[idx] ([P, cols]) in SW-wide chunks."""
                tiles = []
                for h in range(cols // SW):
                    t = pool.tile([P, SW], BF16, tag=tag)
                    eng().dma_start(
                        out=t[:], in_=dram[idx][:, h * SW : (h + 1) * SW]
                    )
                    tiles.append(t)
                return tiles

            # ---------------- Layer 1 ----------------
            with (
                tc.tile_pool(name="xtp", bufs=1) as xt_pool,
                tc.tile_pool(name="w1p", bufs=8) as w1_pool,
            ):
                # x: first tiles in fine chunks so the first matmuls are
                # gated on as little DMA as possible; spread across queues
                xt = [
                    xt_pool.tile([P, bc], BF16, name=f"xt_{k}", tag=f"xt_{k}")
                    for k in range(kt1)
                ]

                def load_x(k, nchunk):
                    cw = bc // nchunk
                    for h in range(nchunk):
                        eng().dma_start(
                            out=xt[k][:, h * cw : (h + 1) * cw],
                            in_=xt_d[k][:, h * cw : (h + 1) * cw],
                        )

                # first x tiles and the first weight panel go at the head of
                # the queues so the PE's first matmuls are gated on minimal DMA
                load_x(0, 4)
                load_x(1, 4)
                w1h0 = load_panel(w1_pool, w1_d, 0, d_in, "w1t")
                nc.sync.dma_start(out=b1_sb[:], in_=b1_d[:])
                nc.gpsimd.dma_start(out=b2_sb[:], in_=b2_d[:])
                nc.sync.dma_start(out=bo_sb[:], in_=bo_d[:])
                for k in range(2, kt1):
                    load_x(k, 2)

                w2pre = None
                for mt in range(mt1):
                    w1h = w1h0 if mt == 0 else load_panel(
                        w1_pool, w1_d, mt, d_in, "w1t"
                    )
                    ps = ps_pool.tile([P, bc], F32, name="ps1", tag="ps")
                    for kt in range(kt1):
                        h, r = divmod(kt, SW // P)
                        for n in range(ns):
                            nc.tensor.matmul(
                                ps[:, n * SW : (n + 1) * SW],
                                w1h[h][:, r * P : (r + 1) * P],
                                xt[kt][:, n * SW : (n + 1) * SW],
                                start=(kt == 0),
                                stop=(kt == kt1 - 1),
                            )
                    nc.scalar.activation(
                        h1[mt][:], ps[:], RELU, bias=b1_sb[:, mt : mt + 1]
                    )
                    if mt == mt1 - 2:
                        # prefetch W2's first panel while L1 finishes
                        w2pre = load_panel(w2_pool, w2_d, 0, d_h, "w2t")

            # ---------------- Layers 2 + 3 ----------------
            with tc.tile_pool(name="wop", bufs=16) as wo_pool:
                wopre = None
                for mt in range(mt2):
                    w2h = w2pre if mt == 0 else load_panel(
                        w2_pool, w2_d, mt, d_h, "w2t"
                    )
                    ps = ps_pool.tile([P, bc], F32, name="ps2", tag="ps")
                    for kt in range(mt1):
                        h, r = divmod(kt, SW // P)
                        for n in range(ns):
                            nc.tensor.matmul(
                                ps[:, n * SW : (n + 1) * SW],
                                w2h[h][:, r * P : (r + 1) * P],
                                h1[kt][:, n * SW : (n + 1) * SW],
                                start=(kt == 0),
                                stop=(kt == mt1 - 1),
                            )
                    nc.scalar.activation(
                        h2[mt][:], ps[:], RELU, bias=b2_sb[:, mt : mt + 1]
                    )
                    if mt == mt2 - 2:
                        # prefetch Wo's first panel while L2 finishes
                        wopre = load_panel(wo_pool, wo_d, 0, d_h, "wot")

                for mo in range(mot):
                    woh = wopre if mo == 0 else load_panel(
                        wo_pool, wo_d, mo, d_h, "wot"
                    )
                    ps = ps_pool.tile([P, bc], F32, name="ps3", tag="ps")
                    for kt in range(mt2):
                        h, r = divmod(kt, SW // P)
                        for n in range(ns):
                            nc.tensor.matmul(
                                ps[:, n * SW : (n + 1) * SW],
                                woh[h][:, r * P : (r + 1) * P],
                                h2[kt][:, n * SW : (n + 1) * SW],
                                start=(kt == 0),
                                stop=(kt == mt2 - 1),
                            )
                    ot = o_pool.tile([P, bc], F32, tag="ot")
                    nc.scalar.activation(
                        ot[:], ps[:], IDENT, bias=bo_sb[:, mo : mo + 1]
                    )
                    nchunk = 8
                    cw = bc // nchunk
                    for h in range(nchunk):
                        eng().dma_start(
                            out=out_d[mo][:, h * cw : (h + 1) * cw],
                            in_=ot[:, h * cw : (h + 1) * cw],
                        )

    nc.compile()
    return nc


def _expand_mask(mask, t=TILE):
    return np.repeat(np.repeat(np.asarray(mask, dtype=bool), t, axis=0), t, axis=1)


def _pack_lhsT(w, d_m, d_k):
    """[d_m, d_k] weights -> [d_m/P, P, d_k] panels.

    panel[mt, i, kt*P + j] = w[mt*P + j, kt*P + i], so each [P, P] slice of a
    panel is a ready-to-use lhsT block (partition dim = contraction dim).
    """
    mt, kt = d_m // P, d_k // P
    return np.ascontiguousarray(
        w.reshape(mt, P, kt, P).transpose(0, 3, 2, 1).reshape(mt, P, d_k)
    )


def _pack_bias(b):
    n = b.shape[0] // P
    return np.ascontiguousarray(b.reshape(n, P).T)


def _run(x, w1e, b1, w2e, b2, wo, bo, d_in, d_h, d_out, n_cores=N_CORES, trace=False):
    b = x.shape[0]
    bc = b // n_cores

    nc = bacc.Bacc(
        "TRN2", target_bir_lowering=False, debug=False, num_devices=n_cores
    )
    _build(nc, d_in, d_h, d_out, bc)

    bf16 = mybir.dt.np(BF16)

    def cvt(a):
        return np.ascontiguousarray(a.astype(bf16))

    shared = {
        "w1": cvt(_pack_lhsT(w1e, d_h, d_in)),
        "b1": _pack_bias(b1),
        "w2": cvt(_pack_lhsT(w2e, d_h, d_h)),
        "b2": _pack_bias(b2),
        "wo": cvt(_pack_lhsT(wo, d_out, d_h)),
        "bo": _pack_bias(bo),
    }
    in_maps = []
    for c in range(n_cores):
        xc = np.ascontiguousarray(x[c * bc : (c + 1) * bc].T).reshape(
            d_in // P, P, bc
        )
        in_maps.append({"xt": cvt(xc), **shared})

    res = run_bass_kernel_spmd(
        nc, in_maps, core_ids=list(range(n_cores)), trace=trace
    )
    outs = []
    for c in range(n_cores):
        outs.append(res.results[c]["out"].reshape(d_out, bc))
    full = np.concatenate(outs, axis=1)  # [d_out, B]
    return np.ascontiguousarray(full.T), res


def kernel(x, W1, b1, W2, b2, Wo, bo, mask1, mask2):
    x = np.asarray(x, dtype=np.float32)
    w1e = np.asarray(W1, dtype=np.float32) * _expand_mask(mask1)
    w2e = np.asarray(W2, dtype=np.float32) * _expand_mask(mask2)
    out, _ = _run(
        x,
        w1e,
        np.asarray(b1, np.float32),
        w2e,
        np.asarray(b2, np.float32),
        np.asarray(Wo, np.float32),
        np.asarray(bo, np.float32),
        d_in=2048,
        d_h=4096,
        d_out=1024,
    )
    return out
